# revision 12
# baseline (speedup 1.0000x reference)
"""Trainium2 Bass kernel for nn_Attention: single-head attention,
B=32, N=1024, DIM=512, fp32.

    q = X @ Wq.T ; k = X @ Wk.T ; v = X @ Wv.T
    out = softmax((q k^T)/sqrt(D)) @ v

Strategy (8 NeuronCores, data-parallel over batch, 4 batches/core):
  - Host folds A = (Wq.T @ Wk)/sqrt(D)  so scores = X A X.T  — saves one
    projection-sized matmul per batch and needs only X (transposed) on
    device.
  - All tensors live transposed on device: XT [d, n], GT = (X A).T,
    V [n, e], scores ST [k, q] (k on partitions).  Softmax runs along
    the partition axis: exp on ScalarE, partition sums via a ones-vector
    matmul, broadcast of 1/denom via a rank-1 ones matmul, normalization
    on VectorE.  Attention output is produced transposed (OT [e, q]) and
    the host transposes it back.
  - Matmuls use the float32r PE mode (full-rate fp32 streaming).
  - Flat software pipeline over h-tiles s = 2b+h (512 q-cols each):
    slot s interleaves S(s) matmuls with O(s-1) matmuls and the gt/v
    projection matmuls of batch b+1; evict/normalize/store of tile s-2
    ride along on Scalar/Vector/DMA.  PE bubbles also reset the PE
    p-state (~3us of half-clock each), so the stream must stay dense.
    The last tile's O phase runs as two 256-wide halves so its norm
    chain overlaps the second half.
"""
import numpy as np

B, N, D = 32, 1024, 512
NCORES = 8
BPC = B // NCORES          # batches per core
DC = D // 128              # 4 chunks of 128 along d / e
KC = N // 128              # 8 chunks of 128 along k
NSLOT = 2 * BPC            # h-tiles per core (512 q-cols each)

_cache = {}


def _split_sync_waits(nc):
    """walrus on this image accepts at most ONE semaphore wait per
    instruction; hoist extras onto InstNoOp carriers on the same engine
    (same-engine program order preserves the gating)."""
    import concourse.mybir as mybir

    ctr = 0
    for f in nc.m.functions:
        for bb in f.blocks:
            out = []
            changed = False
            for ins in bb.instructions:
                si = getattr(ins, "sync_info", None)
                waits = list(si.on_wait) if si and si.on_wait else []
                if len(waits) > 1:
                    for w in waits[:-1]:
                        ctr += 1
                        out.append(
                            mybir.InstNoOp(
                                name=f"wsplit-{ctr}",
                                engine=ins.engine,
                                bass_nofuse=True,
                                sync_info=mybir.SyncInfo(on_wait=[w], on_update=[]),
                            )
                        )
                    ins.sync_info = mybir.SyncInfo(
                        on_wait=waits[-1:], on_update=list(si.on_update or [])
                    )
                    changed = True
                out.append(ins)
            if changed:
                bb.instructions[:] = out


def _build():
    import concourse.bass as bass
    import concourse.mybir as mybir
    import concourse.tile as tile

    f32 = mybir.dt.float32
    f32r = mybir.dt.float32r
    Exp = mybir.ActivationFunctionType.Exp
    Ln = mybir.ActivationFunctionType.Ln

    nc = bass.Bass(target_bir_lowering=False)

    xt = nc.dram_tensor("xt", [BPC, D, N], f32, kind="ExternalInput")
    a_mat = nc.dram_tensor("a_mat", [D, D], f32, kind="ExternalInput")
    wvt = nc.dram_tensor("wvt", [D, D], f32, kind="ExternalInput")
    ones_d = nc.dram_tensor("ones_mat", [128, 128], f32, kind="ExternalInput")
    out_t = nc.dram_tensor("out_t", [BPC, D, N], f32, kind="ExternalOutput")

    with tile.TileContext(nc) as tc:
        with (
            tc.tile_pool(name="wpool", bufs=1) as wpool,
            tc.tile_pool(name="xpool", bufs=3) as xpool,
            tc.tile_pool(name="gpool", bufs=2) as gpool,
            tc.tile_pool(name="vpool", bufs=2) as vpool,
            tc.tile_pool(name="epool", bufs=2) as epool,
            tc.tile_pool(name="eapool", bufs=2) as eapool,
            tc.tile_pool(name="opool", bufs=2) as opool,
            tc.tile_pool(name="rpool", bufs=2) as rpool,
            tc.tile_pool(name="ps_ws", bufs=3, space="PSUM") as ps_ws,
            tc.tile_pool(name="ps_ot", bufs=1, space="PSUM") as ps_ot,
            tc.tile_pool(name="ps_db", bufs=1, space="PSUM") as ps_db,
        ):
            # ---------------- weights / constants (sync queue) ----------
            a_sb = []
            for c in range(DC):
                t = wpool.tile([128, D], f32, tag=f"a{c}", name=f"a_sb{c}")
                nc.sync.dma_start(t[:].bitcast(f32r), a_mat[c * 128:(c + 1) * 128, :].bitcast(f32r))
                a_sb.append(t)
            ones_mat = wpool.tile([128, 128], f32, tag="ones")
            nc.sync.dma_start(ones_mat[:].bitcast(f32r), ones_d[:].bitcast(f32r))
            wvt_sb = [
                wpool.tile([128, D], f32, tag=f"wvt{c}", name=f"wvt_sb{c}")
                for c in range(DC)
            ]

            def load_wvt():
                # rides the scalar queue behind xt0-hh0 so it lands by
                # ~20us for the prologue v(0) groups
                for c in range(DC):
                    nc.scalar.dma_start(
                        wvt_sb[c][:].bitcast(f32r),
                        wvt[c * 128:(c + 1) * 128, :].bitcast(f32r),
                    )

            # ---------------- xt half-tiles (scalar queue) --------------
            # xh[b][k4][hh] = X^T[d-chunk k4, n-half hh]  [128, 512]
            xh = {}

            def load_xt(b):
                tiles = []
                for k4 in range(DC):
                    tiles.append(
                        [
                            xpool.tile(
                                [128, 512], f32, tag=f"x{k4}{hh}",
                                name=f"x_b{b}_{k4}{hh}",
                            )
                            for hh in range(2)
                        ]
                    )
                # hh-outer issue order so the hh=0 set lands first;
                # the two halves ride different queues so they stream in
                # parallel (b=0: hh0 scalar / hh1 sync; b>=1 reversed)
                for hh in range(2):
                    for k4 in range(DC):
                        if b == 0:
                            q = nc.scalar if hh == 0 else nc.gpsimd
                        else:
                            q = nc.sync if hh == 0 else nc.scalar
                        q.dma_start(
                            tiles[k4][hh][:].bitcast(f32r),
                            xt[
                                b, k4 * 128:(k4 + 1) * 128,
                                hh * 512:(hh + 1) * 512,
                            ].bitcast(f32r),
                        )
                xh[b] = tiles

            # xt stationary slice (column chunk kc of N)
            def xslice(b, k4, kc):
                return xh[b][k4][kc // 4][:, (kc % 4) * 128:((kc % 4) + 1) * 128]

            # ---------------- per-batch / per-slot state ----------------
            gt_sb = {}   # [128, DC*N]  G^T chunks at cols m*N + hh*512
            v_sb = {}    # [128, KC*D]  V chunks at cols kc*D
            e_sb = {}    # e_sb[s][kc]  exp tiles [128, 512]
            ea_fin = {}  # final running-sum tile per slot
            rc_sb = {}   # [128, 512] 1/denom broadcast
            otraw = {}   # [128, DC*512] raw O^T per slot
            p_ot = {}    # p_ot[s] = 4 psum accumulators

            # ---------------- emitters ----------------------------------
            def gt_group(bn, m, hh):
                grp = ps_ws.tile([128, 512], f32, tag="ws", name=f"pg{bn}{m}{hh}")
                for k4 in range(DC):
                    nc.tensor.matmul(
                        grp[:],
                        a_sb[k4][:, m * 128:(m + 1) * 128].bitcast(f32r),
                        xh[bn][k4][hh][:].bitcast(f32r),
                        start=(k4 == 0), stop=(k4 == DC - 1),
                    )
                nc.scalar.copy(
                    gt_sb[bn][
                        :, m * N + hh * 512:m * N + (hh + 1) * 512
                    ].bitcast(f32r),
                    grp[:],
                )

            def v_group(bn, kc):
                grp = ps_ws.tile([128, 512], f32, tag="ws", name=f"pv{bn}{kc}")
                for k4 in range(DC):
                    nc.tensor.matmul(
                        grp[:],
                        xslice(bn, k4, kc).bitcast(f32r),
                        wvt_sb[k4][:].bitcast(f32r),
                        start=(k4 == 0), stop=(k4 == DC - 1),
                    )
                nc.scalar.copy(v_sb[bn][:, kc * D:(kc + 1) * D].bitcast(f32r), grp[:])

            def s_group(s, kc, pool=None, tag="ws"):
                b, h = s // 2, s % 2
                p_st = (pool or ps_ws).tile(
                    [128, 512], f32, tag=tag, name=f"st{s}_{kc}"
                )
                for k4 in range(DC):
                    nc.tensor.matmul(
                        p_st[:],
                        xslice(b, k4, kc).bitcast(f32r),
                        gt_sb[b][
                            :, k4 * N + h * 512:k4 * N + (h + 1) * 512
                        ].bitcast(f32r),
                        start=(k4 == 0), stop=(k4 == DC - 1),
                    )
                e = epool.tile([128, 512], f32, tag=f"e{kc}", name=f"e{s}_{kc}")
                nc.scalar.activation(e[:].bitcast(f32r), p_st[:], Exp)
                e_sb[s][kc] = e
                ea = eapool.tile(
                    [128, 512], f32, tag=f"ea{kc % 2}", name=f"ea{s}_{kc}"
                )
                if kc == 0:
                    nc.vector.tensor_copy(ea[:].bitcast(f32r), e[:])
                else:
                    nc.vector.tensor_add(ea[:].bitcast(f32r), ea_fin[s][:], e[:])
                ea_fin[s] = ea

            def o_group(t, kc, lo=0, hi=512, tiles=None):
                """4 O^T matmuls (m inner) accumulating tile t, step kc,
                e-columns lo:hi."""
                bt = t // 2
                tiles = tiles if tiles is not None else p_ot[t]
                for m in range(DC):
                    nc.tensor.matmul(
                        tiles[m][:, 0:hi - lo],
                        v_sb[bt][
                            :, kc * D + m * 128:kc * D + (m + 1) * 128
                        ].bitcast(f32r),
                        e_sb[t][kc][:, lo:hi].bitcast(f32r),
                        start=(kc == 0), stop=(kc == KC - 1),
                    )

            def norm_chain(t, lo=0, hi=512, part=""):
                """denominator broadcast [i,q] = sum_k ea[k,q] via an
                all-ones stationary (fuses the row-sum and the rank-1
                broadcast into one matmul), then 1/x = exp(-ln(x))."""
                w = hi - lo
                p = ps_db.tile([128, 512], f32, tag="db", name=f"pbc{t}{part}")
                nc.tensor.matmul(
                    p[:, 0:w],
                    ones_mat[:].bitcast(f32r),
                    ea_fin[t][:, lo:hi].bitcast(f32r),
                    start=True, stop=True,
                )
                ln = rpool.tile([128, 512], f32, tag="ln", name=f"ln{t}{part}")
                nc.scalar.activation(ln[:, 0:w], p[:, 0:w], Ln)
                if t not in rc_sb:
                    rc_sb[t] = rpool.tile([128, 512], f32, tag="rc", name=f"rc{t}")
                nc.scalar.activation(rc_sb[t][:, lo:hi], ln[:, 0:w], Exp, scale=-1.0)

            def evict_ot(t, m, eng, lo=0, hi=512, tiles=None):
                w = hi - lo
                tiles = tiles if tiles is not None else p_ot[t]
                dst = otraw[t][:, m * 512 + lo:m * 512 + hi]
                if eng == "s":
                    nc.scalar.copy(dst, tiles[m][:, 0:w])
                else:
                    nc.vector.tensor_copy(dst, tiles[m][:, 0:w])

            def mul_ot(t, m, lo=0, hi=512):
                sl = otraw[t][:, m * 512 + lo:m * 512 + hi]
                nc.vector.tensor_mul(sl, sl, rc_sb[t][:, lo:hi])

            def dma_out(t, g, eng, lo=0, hi=512):
                """store one 256-row g-half of tile t (q-cols lo:hi)."""
                b, h = t // 2, t % 2
                dst = out_t[
                    b, g * 256:(g + 1) * 256, h * 512 + lo:h * 512 + hi
                ].rearrange("(m p) q -> p m q", p=128)
                q = nc.scalar if eng == "s" else nc.sync
                if lo == 0 and hi == 512:
                    q.dma_start(
                        dst,
                        otraw[t][:, 2 * g * 512:(2 * g + 2) * 512].rearrange(
                            "p (m q) -> p m q", m=2
                        ),
                    )
                    return
                for j in range(2):
                    m = 2 * g + j
                    q.dma_start(
                        dst[:, j:j + 1, :],
                        otraw[t][:, m * 512 + lo:m * 512 + hi].rearrange(
                            "p (m q) -> p m q", m=1
                        ),
                    )

            def alloc_slot(s):
                e_sb[s] = {}
                otraw[s] = opool.tile(
                    [128, DC * 512], f32, tag="otraw", name=f"orw{s}"
                )
                p_ot[s] = [
                    ps_ot.tile([128, 512], f32, tag=f"ot{m}", name=f"pot{s}{m}")
                    for m in range(DC)
                ]

            # ================= emission ================================
            load_xt(0)
            load_wvt()
            load_xt(1)

            for b in range(BPC):
                gt_sb[b] = gpool.tile([128, DC * N], f32, tag="gt", name=f"gt{b}")
                v_sb[b] = vpool.tile([128, KC * D], f32, tag="v", name=f"v{b}")

            # ---- prologue: gt(0) k4-outer across 8 psum groups
            # (3x ws + 1x db for hh=0; 4x ot for hh=1), consuming xt
            # chunks in DMA-arrival order.  S(0,h0) and v(0) interleave
            # into the DMA-chase gaps so the PE never idles long enough
            # to drop its p-state.
            grp_order = [(m, 0) for m in range(DC)] + [(m, 1) for m in range(DC)]
            grp = {}
            for i, (m, hh) in enumerate(grp_order):
                if i < 3:
                    grp[(m, hh)] = ps_ws.tile(
                        [128, 512], f32, tag="ws", name=f"pg0_{m}{hh}"
                    )
                elif i == 3:
                    grp[(m, hh)] = ps_db.tile(
                        [128, 512], f32, tag="db", name=f"pg0_{m}{hh}"
                    )
                else:
                    grp[(m, hh)] = ps_ot.tile(
                        [128, 512], f32, tag=f"ot{i - 4}", name=f"pg0_{m}{hh}"
                    )

            def gt0_round(k4, hh):
                for m in range(DC):
                    nc.tensor.matmul(
                        grp[(m, hh)][:],
                        a_sb[k4][:, m * 128:(m + 1) * 128].bitcast(f32r),
                        xh[0][k4][hh][:].bitcast(f32r),
                        start=(k4 == 0), stop=(k4 == DC - 1),
                    )

            def gt0_evict(hh):
                # 256-wide halves split across ScalarE/VectorE so all four
                # chunks land ~1.4us after the psum groups stop
                for m in range(DC):
                    base = m * N + hh * 512
                    nc.scalar.copy(
                        gt_sb[0][:, base:base + 256].bitcast(f32r),
                        grp[(m, hh)][:, 0:256],
                    )
                    nc.vector.tensor_copy(
                        gt_sb[0][:, base + 256:base + 512].bitcast(f32r),
                        grp[(m, hh)][:, 256:512],
                    )

            for k4 in range(DC):
                gt0_round(k4, 0)
            gt0_evict(0)
            alloc_slot(0)
            # hh=1 rounds chase the xt0-hh1 chunk arrivals; S(0,h0,kc<4)
            # (which needs only the hh=0 gt chunks) fills the gaps
            for kind, i in (("r", 0), ("r", 1), ("s", 0), ("r", 2),
                            ("s", 1), ("r", 3), ("s", 2), ("s", 3)):
                if kind == "r":
                    gt0_round(i, 1)
                elif i == 3:
                    s_group(0, i, pool=ps_db, tag="db")
                else:
                    s_group(0, i)
            gt0_evict(1)
            # v(0) groups + the rest of S(0,h0)
            for kind, i in (("v", 0), ("v", 1), ("v", 2), ("s", 4),
                            ("v", 3), ("s", 5), ("v", 4), ("s", 6),
                            ("v", 5), ("s", 7), ("v", 6), ("v", 7)):
                if kind == "v":
                    v_group(0, i)
                else:
                    s_group(0, i)

            # ---- steady slots -----------------------------------------
            for s in range(1, NSLOT):
                b, h = s // 2, s % 2
                alloc_slot(s)
                if h == 1 and b + 2 < BPC:
                    load_xt(b + 2)

                for kc in range(KC):
                    if kc == 0 and s >= 2:
                        # free the ot psum banks first thing on Scalar/
                        # Vector so this slot's O groups are not blocked
                        evict_ot(s - 2, 0, "s")
                        evict_ot(s - 2, 1, "v")
                        evict_ot(s - 2, 2, "s")
                        evict_ot(s - 2, 3, "v")

                    # O leads at the last two steps so its psum stop (and
                    # the next slot's evictions) land earlier
                    if kc >= 6:
                        o_group(s - 1, kc)

                    s_group(s, kc)

                    if kc == 1:
                        # kc==1 keeps the norm matmul's counter-based waits
                        # clear of the slot-start Scalar/Vector backlog
                        norm_chain(s - 1)
                    if kc == 3 and s >= 2:
                        for m in range(DC):
                            mul_ot(s - 2, m)
                    if kc == 4 and s >= 2:
                        dma_out(s - 2, 0, "s")
                    if kc == 5 and s >= 2:
                        dma_out(s - 2, 1, "y")

                    if kc < 6:
                        o_group(s - 1, kc)

                    # gt(1) runs in slot 1 (xt(1) has fully landed by then)
                    if s == 1:
                        gt_group(1, kc % 4, kc // 4)
                        v_group(1, kc)
                    elif h == 0 and b >= 1 and b + 1 < BPC:
                        gt_group(b + 1, kc % 4, kc // 4)
                    elif h == 1 and s > 1 and b + 1 < BPC:
                        v_group(b + 1, kc)

            # ---- epilogue: tile L's O phase in two 256-col halves -----
            L = NSLOT - 1
            # free ot banks (tile L-2 was evicted in slot L; L-1 now)
            evict_ot(L - 1, 0, "s")
            evict_ot(L - 1, 1, "v")
            evict_ot(L - 1, 2, "s")
            evict_ot(L - 1, 3, "v")
            for kc in range(KC):
                o_group(L, kc, 0, 256)
                if kc == 1:
                    norm_chain(L, 0, 256, part="a")
                if kc == 2:
                    for m in range(DC):
                        mul_ot(L - 1, m)
                if kc == 3:
                    dma_out(L - 1, 0, "y")
                if kc == 4:
                    dma_out(L - 1, 1, "y")
            # half-a evictions free banks chunk by chunk for half-b
            for m in range(DC):
                evict_ot(L, m, "s" if m % 2 == 0 else "v", 0, 256)
            ot_b = [
                ps_ot.tile([128, 512], f32, tag=f"ot{m}", name=f"potb{m}")
                for m in range(DC)
            ]
            for kc in range(KC):
                o_group(L, kc, 256, 512, tiles=ot_b)
                if kc == 0:
                    norm_chain(L, 256, 512, part="b")
                if kc == 1:
                    for m in range(DC):
                        mul_ot(L, m, 0, 256)
                if kc == 3:
                    dma_out(L, 0, "s", 0, 256)
                if kc == 4:
                    dma_out(L, 1, "y", 0, 256)
            for m in range(DC):
                # fused evict+normalize: multiply PSUM by 1/denom directly
                nc.vector.tensor_mul(
                    otraw[L][:, m * 512 + 256:m * 512 + 512],
                    ot_b[m][:, 0:256],
                    rc_sb[L][:, 256:512],
                )
                g, j = m // 2, m % 2
                q = nc.scalar if m % 2 == 0 else nc.sync
                dst = out_t[
                    L // 2, g * 256:(g + 1) * 256, (L % 2) * 512 + 256:(L % 2) * 512 + 512
                ].rearrange("(m p) q -> p m q", p=128)
                q.dma_start(
                    dst[:, j:j + 1, :],
                    otraw[L][:, m * 512 + 256:m * 512 + 512].rearrange(
                        "p (m q) -> p m q", m=1
                    ),
                )
    return nc


def _prepare_inputs(embeddings, Wq, Wk, Wv):
    xt_all = np.ascontiguousarray(embeddings.transpose(0, 2, 1)).astype(
        np.float32, copy=False
    )
    a_mat = (
        Wq.astype(np.float64).T @ Wk.astype(np.float64) / np.sqrt(float(D))
    ).astype(np.float32)
    wvt = np.ascontiguousarray(Wv.T).astype(np.float32, copy=False)
    ones_mat = np.ones((128, 128), np.float32)
    in_maps = []
    for i in range(NCORES):
        in_maps.append(
            {
                "xt": np.ascontiguousarray(xt_all[i * BPC:(i + 1) * BPC]),
                "a_mat": a_mat,
                "wvt": wvt,
                "ones_mat": ones_mat,
            }
        )
    return in_maps


def _get_nc():
    if "nc" not in _cache:
        nc = _build()
        _split_sync_waits(nc)
        _cache["nc"] = nc
    return _cache["nc"]


def _assemble(results):
    out = np.empty((B, N, D), np.float32)
    for i in range(NCORES):
        ot = results[i]["out_t"]  # [BPC, D, N]
        out[i * BPC:(i + 1) * BPC] = ot.transpose(0, 2, 1)
    return out


def kernel(embeddings, Wq, Wk, Wv):
    from concourse.bass_utils import run_bass_kernel_spmd

    embeddings = np.asarray(embeddings, dtype=np.float32)
    in_maps = _prepare_inputs(
        embeddings, np.asarray(Wq), np.asarray(Wk), np.asarray(Wv)
    )
    res = run_bass_kernel_spmd(_get_nc(), in_maps, list(range(NCORES)))
    return _assemble(res.results)
